# revision 1
# baseline (speedup 1.0000x reference)
"""Trainium2 Bass kernel for the CurrentLIFNetwork problem.

Strategy (8 NeuronCores, tensor-parallel over the recurrent matrix):
  - Core c owns output neurons [c*512, (c+1)*512). Its W shard
    (4096 x 512, pre-scaled by the E/I column scaling) and feedforward
    shard (512 x 512) live SBUF-resident for the whole run.
  - V/I state is kept per-core in a [128 partitions, 4 chunks, 8 batch]
    layout (neuron n_local = chunk*128 + p), which doubles as the
    transposed-spike (s^T) layout the TensorEngine needs for lhsT.
  - Per step: DVE computes u = v-u_rest update + spikes; spikes are
    DMA'd to a DRAM stage buffer; an 8-core AllGather produces the full
    4096-neuron spike vector; 32 accumulating matmuls (lhsT = gathered
    s^T chunk [128,8], rhs = W chunk [128,512]) plus 4 feedforward
    matmuls (on the prefetched input column for this step) produce
    I_delta in PSUM [8,512]; DVE 32x32 stream-transposes bring it back
    to the state layout; I is updated; s/I/v are staged and written to
    DRAM output.
  - The feedforward matmuls only depend on the (prefetched) input, so
    they are issued before the gather-dependent matmuls and overlap the
    collective latency.

Everything (shapes, sharding) is hardcoded for the problem instance:
B=8, N=4096, N_IN=512, n_cores=8; T is read from n_steps.
"""
import sys
import time

sys.path.insert(0, "/opt/trn_rl_repo")

import numpy as np

import concourse.bacc as bacc
import concourse.bass as bass
import concourse.mybir as mybir
import concourse.tile as tile
from concourse import bass2jax
from concourse.bass2jax import _bass_exec_p, install_neuronx_cc_hook, partition_id_tensor

F32 = mybir.dt.float32

# physiological constants (mirror reference.py)
DT = 1.0
BETA_E = float(np.exp(-DT / 20.0))
BETA_I = float(np.exp(-DT / 10.0))
ALPHA_E = float(np.exp(-DT / 5.0))
ALPHA_I = float(np.exp(-DT / 5.0))
R_E, R_I = 100.0, 100.0
U_REST = -65.0
THETA = -50.0
U_RESET = -65.0
E_WEIGHT, I_WEIGHT = 0.5, 2.0

NCORES = 8
B = 8
N = 4096
NIN = 512
NLOC = N // NCORES          # 512 neurons per core
NCH = NLOC // 128           # 4 local chunks
KCH = N // 128              # 32 global chunks
RG = [list(range(NCORES))]


def build_lif_kernel(T: int):
    nc = bacc.Bacc("TRN2", target_bir_lowering=False, debug=False, num_devices=NCORES)

    w_in = nc.dram_tensor("w", [KCH, 128, NLOC], F32, kind="ExternalInput")
    f_in = nc.dram_tensor("f", [NCH, 128, NLOC], F32, kind="ExternalInput")
    x_in = nc.dram_tensor("x", [T, NCH, 128, B], F32, kind="ExternalInput")
    cb_in = nc.dram_tensor("cb", [NCH, 128, B], F32, kind="ExternalInput")
    cc_in = nc.dram_tensor("cc", [NCH, 128, B], F32, kind="ExternalInput")
    cal_in = nc.dram_tensor("cal", [NCH, 128, B], F32, kind="ExternalInput")
    u0_in = nc.dram_tensor("u0", [NCH, 128, B], F32, kind="ExternalInput")
    i0_in = nc.dram_tensor("i0", [NCH, 128, B], F32, kind="ExternalInput")
    out_d = nc.dram_tensor("out", [T, 3, NCH, 128, B], F32, kind="ExternalOutput")

    stage = [nc.dram_tensor(f"stage{i}", [NCH, 128, B], F32) for i in range(2)]
    gath = [nc.dram_tensor(f"gath{i}", [KCH, 128, B], F32) for i in range(2)]

    with tile.TileContext(nc) as tc:
        with (
            tc.tile_pool(name="wpool", bufs=1) as wpool,
            tc.tile_pool(name="state", bufs=1) as spool,
            tc.tile_pool(name="work", bufs=3) as wk,
            tc.tile_pool(name="xp", bufs=6) as xp,
            tc.tile_pool(name="gp", bufs=2) as gp,
            tc.tile_pool(name="op", bufs=3) as op,
            tc.tile_pool(name="ps", bufs=2, space="PSUM") as ps,
        ):
            w_sb = wpool.tile([128, KCH, NLOC], F32)
            f_sb = wpool.tile([128, NCH, NLOC], F32)
            cb_sb = wpool.tile([128, NCH, B], F32)
            cc_sb = wpool.tile([128, NCH, B], F32)
            cal_sb = wpool.tile([128, NCH, B], F32)
            nc.scalar.dma_start(out=w_sb[:], in_=w_in.ap().rearrange("k p n -> p k n"))
            nc.scalar.dma_start(out=f_sb[:], in_=f_in.ap().rearrange("k p n -> p k n"))
            nc.scalar.dma_start(out=cb_sb[:], in_=cb_in.ap().rearrange("k p b -> p k b"))
            nc.scalar.dma_start(out=cc_sb[:], in_=cc_in.ap().rearrange("k p b -> p k b"))
            nc.scalar.dma_start(out=cal_sb[:], in_=cal_in.ap().rearrange("k p b -> p k b"))

            u_sb = spool.tile([128, NCH, B], F32)
            i_sb = spool.tile([128, NCH, B], F32)
            nc.scalar.dma_start(out=u_sb[:], in_=u0_in.ap().rearrange("k p b -> p k b"))
            nc.scalar.dma_start(out=i_sb[:], in_=i0_in.ap().rearrange("k p b -> p k b"))

            for t in range(T):
                p = t % 2
                # prefetch this step's input column (ACT HWDGE ring)
                x_sb = xp.tile([128, NCH, B], F32)
                nc.scalar.dma_start(
                    out=x_sb[:], in_=x_in.ap()[t].rearrange("k p b -> p k b")
                )

                # ---- membrane update + spikes (DVE) ----
                t1 = wk.tile([128, NCH, B], F32)
                t2 = wk.tile([128, NCH, B], F32)
                upre = wk.tile([128, NCH, B], F32)
                stage_all = op.tile([128, 3, NCH, B], F32)
                s_t = stage_all[:, 0]
                nc.vector.tensor_mul(t1[:], u_sb[:], cb_sb[:])
                nc.vector.tensor_mul(t2[:], i_sb[:], cc_sb[:])
                nc.vector.tensor_add(upre[:], t1[:], t2[:])
                nc.vector.tensor_scalar(
                    s_t, upre[:], THETA - U_REST, None, mybir.AluOpType.is_ge
                )
                # u after reset: keep where no spike
                nc.vector.scalar_tensor_tensor(
                    u_sb[:], s_t, 0.0, upre[:],
                    mybir.AluOpType.is_equal, mybir.AluOpType.mult,
                )

                # ---- stage spikes + AllGather + load gathered ----
                nc.sync.dma_start(
                    out=stage[p].ap().rearrange("k q b -> q k b"), in_=s_t
                )
                nc.gpsimd.collective_compute(
                    "AllGather", mybir.AluOpType.bypass, replica_groups=RG,
                    ins=[stage[p].ap().opt()], outs=[gath[p].ap().opt()],
                )
                g_sb = gp.tile([128, KCH, B], F32)
                nc.sync.dma_start(
                    out=g_sb[:], in_=gath[p].ap().rearrange("k q b -> q k b")
                )

                # ---- matmuls: drive (input) first, then recurrent ----
                pb = ps.tile([32, NLOC], F32)
                for k in range(NCH):
                    nc.tensor.matmul(
                        pb[0:B, :], x_sb[:, k], f_sb[:, k],
                        start=(k == 0), stop=False,
                    )
                for k in range(KCH):
                    nc.tensor.matmul(
                        pb[0:B, :], g_sb[:, k], w_sb[:, k],
                        start=False, stop=(k == KCH - 1),
                    )

                # ---- transpose PSUM [8,512] back to state layout ----
                delta = wk.tile([128, NCH, 32], F32)
                pview = pb[0:32, :].rearrange("q (c g s) -> q c g s", g=NCH, s=32)
                for g4 in range(4):
                    nc.vector.transpose(delta[32 * g4:32 * (g4 + 1)], pview[:, :, g4])

                # ---- I update ----
                t3 = wk.tile([128, NCH, B], F32)
                nc.vector.tensor_mul(t3[:], i_sb[:], cal_sb[:])
                nc.vector.tensor_add(i_sb[:], t3[:], delta[:, :, 0:B])

                # ---- stage outputs (I, v) and write out ----
                nc.scalar.copy(stage_all[:, 1], i_sb[:])
                nc.vector.tensor_scalar_add(stage_all[:, 2], u_sb[:], U_REST)
                nc.gpsimd.dma_start(
                    out=out_d.ap()[t].rearrange("c k q b -> q c k b"),
                    in_=stage_all[:],
                )
    nc.compile()
    return nc


class SpmdRunner:
    """jit once, execute many times; warm runs measure device time + dispatch."""

    def __init__(self, nc: bass.Bass, n_cores: int = NCORES):
        import jax
        from jax.sharding import Mesh, PartitionSpec
        from jax.experimental.shard_map import shard_map

        install_neuronx_cc_hook()
        self.jax = jax
        self.nc = nc
        self.n_cores = n_cores
        partition_name = nc.partition_id_tensor.name if nc.partition_id_tensor else None
        dbg_name = nc.dbg_addr.name if nc.dbg_addr else None
        in_names, out_names, out_avals, zero_outs = [], [], [], []
        for alloc in nc.m.functions[0].allocations:
            if not isinstance(alloc, mybir.MemoryLocationSet):
                continue
            name = alloc.memorylocations[0].name
            if alloc.kind == "ExternalInput":
                if name not in (partition_name, dbg_name):
                    in_names.append(name)
            elif alloc.kind == "ExternalOutput":
                out_names.append(name)
                shape = tuple(alloc.tensor_shape)
                dtype = mybir.dt.np(alloc.dtype)
                out_avals.append(jax.core.ShapedArray(shape, dtype))
                zero_outs.append(np.zeros(shape, dtype))
        self.in_names, self.out_names = in_names, out_names
        self.zero_outs = zero_outs
        n_params, n_outs = len(in_names), len(out_avals)
        all_in_names = list(in_names) + list(out_names)
        if dbg_name:
            all_in_names.append(dbg_name)
        if partition_name is not None:
            all_in_names.append(partition_name)

        def _body(*args):
            operands = list(args)
            if dbg_name:
                operands.append(jax.numpy.zeros((1, 2), jax.numpy.uint32))
            if partition_name is not None:
                operands.append(partition_id_tensor())
            outs = _bass_exec_p.bind(
                *operands,
                out_avals=tuple(out_avals),
                in_names=tuple(all_in_names),
                out_names=tuple(out_names),
                lowering_input_output_aliases=(),
                sim_require_finite=True,
                sim_require_nnan=True,
                nc=nc,
            )
            return tuple(outs)

        devices = jax.devices()[:n_cores]
        mesh = Mesh(np.asarray(devices), ("core",))
        in_specs = (PartitionSpec("core"),) * (n_params + n_outs)
        out_specs = (PartitionSpec("core"),) * n_outs
        self.fn = jax.jit(
            shard_map(_body, mesh=mesh, in_specs=in_specs, out_specs=out_specs,
                      check_rep=False),
            keep_unused=True,
        )

    def prepare(self, in_maps):
        concat = [
            np.concatenate([np.asarray(m[name]) for m in in_maps], axis=0)
            for name in self.in_names
        ]
        concat += [np.concatenate([z] * self.n_cores, axis=0) for z in self.zero_outs]
        return concat

    def run(self, concat_in):
        outs = self.fn(*concat_in)
        self.jax.block_until_ready(outs)
        return outs

    def split_outs(self, outs):
        results = [dict() for _ in range(self.n_cores)]
        for name, arr in zip(self.out_names, outs):
            arr = np.asarray(arr)
            per = arr.shape[0] // self.n_cores
            for c in range(self.n_cores):
                results[c][name] = arr[c * per:(c + 1) * per]
        return results


def _prep_inputs(neuron_types, recurrent_weights, feedforward_weights, inputs,
                 initial_v, initial_I, T):
    is_exc = (np.asarray(neuron_types) == 1)
    beta = np.where(is_exc, BETA_E, BETA_I).astype(np.float32)
    alpha = np.where(is_exc, ALPHA_E, ALPHA_I).astype(np.float32)
    Cgain = (R_E * (1.0 - beta)).astype(np.float32)
    scaling = np.where(is_exc, E_WEIGHT, I_WEIGHT).astype(np.float32)
    Wp = np.asarray(recurrent_weights, np.float32) * scaling[None, :]
    F = np.asarray(feedforward_weights, np.float32)
    xT = np.ascontiguousarray(
        np.asarray(inputs, np.float32).transpose(1, 2, 0)
    ).reshape(T, NCH, 128, B)
    u0 = np.ascontiguousarray(
        (np.asarray(initial_v, np.float32) - U_REST).T
    )  # (4096, 8)
    i0 = np.ascontiguousarray(np.asarray(initial_I, np.float32).T)

    def pc(vec):  # per-core const (4,128) -> (4,128,8)
        return np.ascontiguousarray(
            np.broadcast_to(vec.reshape(NCORES, NCH, 128, 1), (NCORES, NCH, 128, B))
        ).astype(np.float32)

    cb, cal, ccg = pc(beta), pc(alpha), pc(Cgain)
    in_maps = []
    for c in range(NCORES):
        cols = slice(c * NLOC, (c + 1) * NLOC)
        in_maps.append({
            "w": np.ascontiguousarray(Wp[:, cols]).reshape(KCH, 128, NLOC),
            "f": np.ascontiguousarray(F[:, cols]).reshape(NCH, 128, NLOC),
            "x": xT,
            "cb": cb[c], "cc": ccg[c], "cal": cal[c],
            "u0": u0[cols].reshape(NCH, 128, B),
            "i0": i0[cols].reshape(NCH, 128, B),
        })
    return in_maps


def _unshard(results, T):
    outs = []
    for ch in range(3):
        parts = []
        for c in range(NCORES):
            a = results[c]["out"][:, ch]  # (T, 4, 128, 8)
            parts.append(a.reshape(T, NLOC, B).transpose(2, 0, 1))
        outs.append(np.concatenate(parts, axis=2))  # (8, T, 4096)
    return outs[0], outs[2], outs[1]  # s, v, I


_CACHE = {}


def _get_runner(T):
    if T not in _CACHE:
        nc = build_lif_kernel(T)
        _CACHE[T] = SpmdRunner(nc, NCORES)
    return _CACHE[T]


def kernel(neuron_types, recurrent_weights, feedforward_weights, inputs,
           initial_v, initial_I, n_steps):
    T = int(n_steps)
    assert np.asarray(inputs).shape == (B, T, NIN)
    in_maps = _prep_inputs(neuron_types, recurrent_weights, feedforward_weights,
                           inputs, initial_v, initial_I, T)
    runner = _get_runner(T)
    concat_in = runner.prepare(in_maps)
    outs = runner.run(concat_in)
    results = runner.split_outs(outs)
    s, v, I = _unshard(results, T)
    return s, v, I


# revision 2
# speedup vs baseline: 8.6871x; 8.6871x over previous
"""Trainium2 Bass kernel for the CurrentLIFNetwork problem.

Strategy (8 NeuronCores, tensor-parallel over the recurrent matrix):
  - Core c owns output neurons [c*512, (c+1)*512). Its W shard
    (4096 x 512, pre-scaled by the E/I column scaling) and feedforward
    shard (512 x 512) live SBUF-resident for the whole run.
  - V/I state is kept per-core in a [128 partitions, 4 chunks, 8 batch]
    layout (neuron n_local = chunk*128 + p), which doubles as the
    transposed-spike (s^T) layout the TensorEngine needs for lhsT.
  - Per step: DVE computes u = v-u_rest update + spikes; spikes are
    DMA'd to a DRAM stage buffer; an 8-core AllGather produces the full
    4096-neuron spike vector; 32 accumulating matmuls (lhsT = gathered
    s^T chunk [128,8], rhs = W chunk [128,512]) plus 4 feedforward
    matmuls (on the prefetched input column for this step) produce
    I_delta in PSUM [8,512]; DVE 32x32 stream-transposes bring it back
    to the state layout; I is updated; s/I/v are staged and written to
    DRAM output.
  - The feedforward matmuls only depend on the (prefetched) input, so
    they are issued before the gather-dependent matmuls and overlap the
    collective latency.

Everything (shapes, sharding) is hardcoded for the problem instance:
B=8, N=4096, N_IN=512, n_cores=8; T is read from n_steps.
"""
import sys
import time

sys.path.insert(0, "/opt/trn_rl_repo")

import numpy as np

import concourse.bacc as bacc
import concourse.bass as bass
import concourse.mybir as mybir
import concourse.tile as tile
from concourse import bass2jax
from concourse.bass2jax import _bass_exec_p, install_neuronx_cc_hook, partition_id_tensor

F32 = mybir.dt.float32

# physiological constants (mirror reference.py)
DT = 1.0
BETA_E = float(np.exp(-DT / 20.0))
BETA_I = float(np.exp(-DT / 10.0))
ALPHA_E = float(np.exp(-DT / 5.0))
ALPHA_I = float(np.exp(-DT / 5.0))
R_E, R_I = 100.0, 100.0
U_REST = -65.0
THETA = -50.0
U_RESET = -65.0
E_WEIGHT, I_WEIGHT = 0.5, 2.0

NCORES = 8
B = 8
N = 4096
NIN = 512
NLOC = N // NCORES          # 512 neurons per core
NCH = NLOC // 128           # 4 local chunks
KCH = N // 128              # 32 global chunks
RG = [list(range(NCORES))]


def build_lif_kernel(T: int):
    nc = bacc.Bacc("TRN2", target_bir_lowering=False, debug=False, num_devices=NCORES)

    w_in = nc.dram_tensor("w", [KCH, 128, NLOC], F32, kind="ExternalInput")
    f_in = nc.dram_tensor("f", [NCH, 128, NLOC], F32, kind="ExternalInput")
    x_in = nc.dram_tensor("x", [T, NCH, 128, B], F32, kind="ExternalInput")
    cb_in = nc.dram_tensor("cb", [NCH, 128, B], F32, kind="ExternalInput")
    cc_in = nc.dram_tensor("cc", [NCH, 128, B], F32, kind="ExternalInput")
    cal_in = nc.dram_tensor("cal", [NCH, 128, B], F32, kind="ExternalInput")
    u0_in = nc.dram_tensor("u0", [NCH, 128, B], F32, kind="ExternalInput")
    i0_in = nc.dram_tensor("i0", [NCH, 128, B], F32, kind="ExternalInput")
    out_d = nc.dram_tensor("out", [T, 3, NCH, 128, B], F32, kind="ExternalOutput")

    stage = [nc.dram_tensor(f"stage{i}", [NCH, 128, B], F32) for i in range(2)]
    gath = [nc.dram_tensor(f"gath{i}", [KCH, 128, B], F32) for i in range(2)]

    with tile.TileContext(nc) as tc:
        with (
            tc.tile_pool(name="wpool", bufs=1) as wpool,
            tc.tile_pool(name="state", bufs=1) as spool,
            tc.tile_pool(name="work", bufs=3) as wk,
            tc.tile_pool(name="xp", bufs=6) as xp,
            tc.tile_pool(name="gp", bufs=2) as gp,
            tc.tile_pool(name="op", bufs=3) as op,
            tc.tile_pool(name="ps", bufs=2, space="PSUM") as ps,
        ):
            w_sb = wpool.tile([128, KCH, NLOC], F32)
            f_sb = wpool.tile([128, NCH, NLOC], F32)
            cb_sb = wpool.tile([128, NCH, B], F32)
            cc_sb = wpool.tile([128, NCH, B], F32)
            cal_sb = wpool.tile([128, NCH, B], F32)
            nc.scalar.dma_start(out=w_sb[:], in_=w_in.ap().rearrange("k p n -> p k n"))
            nc.scalar.dma_start(out=f_sb[:], in_=f_in.ap().rearrange("k p n -> p k n"))
            nc.scalar.dma_start(out=cb_sb[:], in_=cb_in.ap().rearrange("k p b -> p k b"))
            nc.scalar.dma_start(out=cc_sb[:], in_=cc_in.ap().rearrange("k p b -> p k b"))
            nc.scalar.dma_start(out=cal_sb[:], in_=cal_in.ap().rearrange("k p b -> p k b"))

            u_sb = spool.tile([128, NCH, B], F32)
            i_sb = spool.tile([128, NCH, B], F32)
            nc.scalar.dma_start(out=u_sb[:], in_=u0_in.ap().rearrange("k p b -> p k b"))
            nc.scalar.dma_start(out=i_sb[:], in_=i0_in.ap().rearrange("k p b -> p k b"))

            for t in range(T):
                p = t % 2
                # prefetch this step's input column (ACT HWDGE ring)
                x_sb = xp.tile([128, NCH, B], F32)
                nc.scalar.dma_start(
                    out=x_sb[:], in_=x_in.ap()[t].rearrange("k p b -> p k b")
                )

                # ---- membrane update + spikes (DVE) ----
                t1 = wk.tile([128, NCH, B], F32)
                t2 = wk.tile([128, NCH, B], F32)
                upre = wk.tile([128, NCH, B], F32)
                stage_all = op.tile([128, 3, NCH, B], F32)
                s_t = stage_all[:, 0]
                nc.vector.tensor_mul(t1[:], u_sb[:], cb_sb[:])
                nc.vector.tensor_mul(t2[:], i_sb[:], cc_sb[:])
                nc.vector.tensor_add(upre[:], t1[:], t2[:])
                nc.vector.tensor_scalar(
                    s_t, upre[:], THETA - U_REST, None, mybir.AluOpType.is_ge
                )
                # u after reset: keep where no spike
                nc.vector.scalar_tensor_tensor(
                    u_sb[:], s_t, 0.0, upre[:],
                    mybir.AluOpType.is_equal, mybir.AluOpType.mult,
                )

                # ---- stage spikes + AllGather + load gathered ----
                nc.sync.dma_start(
                    out=stage[p].ap().rearrange("k q b -> q k b"), in_=s_t
                )
                nc.gpsimd.collective_compute(
                    "AllGather", mybir.AluOpType.bypass, replica_groups=RG,
                    ins=[stage[p].ap().opt()], outs=[gath[p].ap().opt()],
                )
                g_sb = gp.tile([128, KCH, B], F32)
                nc.sync.dma_start(
                    out=g_sb[:], in_=gath[p].ap().rearrange("k q b -> q k b")
                )

                # ---- matmuls: drive (input) first, then recurrent ----
                pb = ps.tile([32, NLOC], F32)
                for k in range(NCH):
                    nc.tensor.matmul(
                        pb[0:B, :], x_sb[:, k], f_sb[:, k],
                        start=(k == 0), stop=False,
                    )
                for k in range(KCH):
                    nc.tensor.matmul(
                        pb[0:B, :], g_sb[:, k], w_sb[:, k],
                        start=False, stop=(k == KCH - 1),
                    )

                # ---- transpose PSUM [8,512] back to state layout ----
                delta = wk.tile([128, NCH, 32], F32)
                pview = pb[0:32, :].rearrange("q (c g s) -> q c g s", g=NCH, s=32)
                for g4 in range(4):
                    nc.vector.transpose(delta[32 * g4:32 * (g4 + 1)], pview[:, :, g4])

                # ---- I update ----
                t3 = wk.tile([128, NCH, B], F32)
                nc.vector.tensor_mul(t3[:], i_sb[:], cal_sb[:])
                nc.vector.tensor_add(i_sb[:], t3[:], delta[:, :, 0:B])

                # ---- stage outputs (I, v) and write out ----
                nc.scalar.copy(stage_all[:, 1], i_sb[:])
                nc.vector.tensor_scalar_add(stage_all[:, 2], u_sb[:], U_REST)
                nc.gpsimd.dma_start(
                    out=out_d.ap()[t].rearrange("c k q b -> q c k b"),
                    in_=stage_all[:],
                )
    nc.compile()
    return nc


class SpmdRunner:
    """jit once, execute many times; warm runs measure device time + dispatch."""

    def __init__(self, nc: bass.Bass, n_cores: int = NCORES):
        import jax
        from jax.sharding import Mesh, PartitionSpec
        from jax.experimental.shard_map import shard_map

        install_neuronx_cc_hook()
        self.jax = jax
        self.nc = nc
        self.n_cores = n_cores
        partition_name = nc.partition_id_tensor.name if nc.partition_id_tensor else None
        dbg_name = nc.dbg_addr.name if nc.dbg_addr else None
        in_names, out_names, out_avals, zero_outs = [], [], [], []
        for alloc in nc.m.functions[0].allocations:
            if not isinstance(alloc, mybir.MemoryLocationSet):
                continue
            name = alloc.memorylocations[0].name
            if alloc.kind == "ExternalInput":
                if name not in (partition_name, dbg_name):
                    in_names.append(name)
            elif alloc.kind == "ExternalOutput":
                out_names.append(name)
                shape = tuple(alloc.tensor_shape)
                dtype = mybir.dt.np(alloc.dtype)
                out_avals.append(jax.core.ShapedArray(shape, dtype))
                zero_outs.append(np.zeros(shape, dtype))
        self.in_names, self.out_names = in_names, out_names
        self.zero_outs = zero_outs
        n_params, n_outs = len(in_names), len(out_avals)
        all_in_names = list(in_names) + list(out_names)
        if dbg_name:
            all_in_names.append(dbg_name)
        if partition_name is not None:
            all_in_names.append(partition_name)

        def _body(*args):
            operands = list(args)
            if dbg_name:
                operands.append(jax.numpy.zeros((1, 2), jax.numpy.uint32))
            if partition_name is not None:
                operands.append(partition_id_tensor())
            outs = _bass_exec_p.bind(
                *operands,
                out_avals=tuple(out_avals),
                in_names=tuple(all_in_names),
                out_names=tuple(out_names),
                lowering_input_output_aliases=(),
                sim_require_finite=True,
                sim_require_nnan=True,
                nc=nc,
            )
            return tuple(outs)

        devices = jax.devices()[:n_cores]
        mesh = Mesh(np.asarray(devices), ("core",))
        in_specs = (PartitionSpec("core"),) * (n_params + n_outs)
        out_specs = (PartitionSpec("core"),) * n_outs
        self.fn = jax.jit(
            shard_map(_body, mesh=mesh, in_specs=in_specs, out_specs=out_specs,
                      check_rep=False),
            keep_unused=True,
        )

    def prepare(self, in_maps):
        from jax.sharding import Mesh, NamedSharding, PartitionSpec

        jax = self.jax
        concat = [
            np.concatenate([np.asarray(m[name]) for m in in_maps], axis=0)
            for name in self.in_names
        ]
        concat += [np.concatenate([z] * self.n_cores, axis=0) for z in self.zero_outs]
        mesh = Mesh(np.asarray(jax.devices()[:self.n_cores]), ("core",))
        sh = NamedSharding(mesh, PartitionSpec("core"))
        placed = [jax.device_put(a, sh) for a in concat]
        jax.block_until_ready(placed)
        return placed

    def run(self, concat_in):
        outs = self.fn(*concat_in)
        self.jax.block_until_ready(outs)
        return outs

    def split_outs(self, outs):
        results = [dict() for _ in range(self.n_cores)]
        for name, arr in zip(self.out_names, outs):
            arr = np.asarray(arr)
            per = arr.shape[0] // self.n_cores
            for c in range(self.n_cores):
                results[c][name] = arr[c * per:(c + 1) * per]
        return results


def _prep_inputs(neuron_types, recurrent_weights, feedforward_weights, inputs,
                 initial_v, initial_I, T):
    is_exc = (np.asarray(neuron_types) == 1)
    beta = np.where(is_exc, BETA_E, BETA_I).astype(np.float32)
    alpha = np.where(is_exc, ALPHA_E, ALPHA_I).astype(np.float32)
    Cgain = (R_E * (1.0 - beta)).astype(np.float32)
    scaling = np.where(is_exc, E_WEIGHT, I_WEIGHT).astype(np.float32)
    Wp = np.asarray(recurrent_weights, np.float32) * scaling[None, :]
    F = np.asarray(feedforward_weights, np.float32)
    xT = np.ascontiguousarray(
        np.asarray(inputs, np.float32).transpose(1, 2, 0)
    ).reshape(T, NCH, 128, B)
    u0 = np.ascontiguousarray(
        (np.asarray(initial_v, np.float32) - U_REST).T
    )  # (4096, 8)
    i0 = np.ascontiguousarray(np.asarray(initial_I, np.float32).T)

    def pc(vec):  # per-core const (4,128) -> (4,128,8)
        return np.ascontiguousarray(
            np.broadcast_to(vec.reshape(NCORES, NCH, 128, 1), (NCORES, NCH, 128, B))
        ).astype(np.float32)

    cb, cal, ccg = pc(beta), pc(alpha), pc(Cgain)
    in_maps = []
    for c in range(NCORES):
        cols = slice(c * NLOC, (c + 1) * NLOC)
        in_maps.append({
            "w": np.ascontiguousarray(Wp[:, cols]).reshape(KCH, 128, NLOC),
            "f": np.ascontiguousarray(F[:, cols]).reshape(NCH, 128, NLOC),
            "x": xT,
            "cb": cb[c], "cc": ccg[c], "cal": cal[c],
            "u0": u0[cols].reshape(NCH, 128, B),
            "i0": i0[cols].reshape(NCH, 128, B),
        })
    return in_maps


def _unshard(results, T):
    outs = []
    for ch in range(3):
        parts = []
        for c in range(NCORES):
            a = results[c]["out"][:, ch]  # (T, 4, 128, 8)
            parts.append(a.reshape(T, NLOC, B).transpose(2, 0, 1))
        outs.append(np.concatenate(parts, axis=2))  # (8, T, 4096)
    return outs[0], outs[2], outs[1]  # s, v, I


_CACHE = {}


def _get_runner(T):
    if T not in _CACHE:
        nc = build_lif_kernel(T)
        _CACHE[T] = SpmdRunner(nc, NCORES)
    return _CACHE[T]


def kernel(neuron_types, recurrent_weights, feedforward_weights, inputs,
           initial_v, initial_I, n_steps):
    T = int(n_steps)
    assert np.asarray(inputs).shape == (B, T, NIN)
    in_maps = _prep_inputs(neuron_types, recurrent_weights, feedforward_weights,
                           inputs, initial_v, initial_I, T)
    runner = _get_runner(T)
    concat_in = runner.prepare(in_maps)
    outs = runner.run(concat_in)
    results = runner.split_outs(outs)
    s, v, I = _unshard(results, T)
    return s, v, I


# revision 10
# speedup vs baseline: 10.7461x; 1.2370x over previous
"""Trainium2 Bass kernel for the CurrentLIFNetwork problem.

Strategy (8 NeuronCores, tensor-parallel over the recurrent matrix):
  - Core c owns output neurons [c*512, (c+1)*512). Its W shard
    (4096 x 512, pre-scaled by the E/I column scaling) and feedforward
    shard (512 x 512) live SBUF-resident for the whole run.
  - V/I state is kept per-core in a [128 partitions, 4 chunks, 8 batch]
    layout (neuron n_local = chunk*128 + p), which doubles as the
    transposed-spike (s^T) layout the TensorEngine needs for lhsT.
  - Per step: DVE computes u = v-u_rest update + spikes; spikes are
    DMA'd to a DRAM stage buffer; an 8-core AllGather produces the full
    4096-neuron spike vector; 32 accumulating matmuls (lhsT = gathered
    s^T chunk [128,8], rhs = W chunk [128,512]) plus 4 feedforward
    matmuls (on the prefetched input column for this step) produce
    I_delta in PSUM [8,512]; DVE 32x32 stream-transposes bring it back
    to the state layout; I is updated; s/I/v are staged and written to
    DRAM output.
  - The feedforward matmuls only depend on the (prefetched) input, so
    they are issued before the gather-dependent matmuls and overlap the
    collective latency.

Everything (shapes, sharding) is hardcoded for the problem instance:
B=8, N=4096, N_IN=512, n_cores=8; T is read from n_steps.
"""
import sys
import time

sys.path.insert(0, "/opt/trn_rl_repo")

import numpy as np

import concourse.bacc as bacc
import concourse.bass as bass
import concourse.mybir as mybir
import concourse.tile as tile
from concourse import bass2jax
from concourse.bass2jax import _bass_exec_p, install_neuronx_cc_hook, partition_id_tensor

F32 = mybir.dt.float32

# physiological constants (mirror reference.py)
DT = 1.0
BETA_E = float(np.exp(-DT / 20.0))
BETA_I = float(np.exp(-DT / 10.0))
ALPHA_E = float(np.exp(-DT / 5.0))
ALPHA_I = float(np.exp(-DT / 5.0))
R_E, R_I = 100.0, 100.0
U_REST = -65.0
THETA = -50.0
U_RESET = -65.0
E_WEIGHT, I_WEIGHT = 0.5, 2.0

NCORES = 8
B = 8
N = 4096
NIN = 512
NLOC = N // NCORES          # 512 neurons per core
NCH = NLOC // 128           # 4 local chunks
KCH = N // 128              # 32 global chunks
RG = [list(range(NCORES))]


def build_lif_kernel(T: int):
    nc = bacc.Bacc("TRN2", target_bir_lowering=False, debug=False, num_devices=NCORES)

    w_in = nc.dram_tensor("w", [KCH, 128, NLOC], F32, kind="ExternalInput")
    f_in = nc.dram_tensor("f", [NCH, 128, NLOC], F32, kind="ExternalInput")
    # x is padded to 32 batch columns so the start=True feedforward matmul
    # initializes its full 32-partition PSUM stripe (cols 8:32 are zeros)
    x_in = nc.dram_tensor("x", [T, NCH, 128, 32], F32, kind="ExternalInput")
    js_in = nc.dram_tensor("jsel", [128, B], F32, kind="ExternalInput")
    cb_in = nc.dram_tensor("cb", [NCH, 128, B], F32, kind="ExternalInput")
    cc_in = nc.dram_tensor("cc", [NCH, 128, B], F32, kind="ExternalInput")
    cal_in = nc.dram_tensor("cal", [NCH, 128, B], F32, kind="ExternalInput")
    u0_in = nc.dram_tensor("u0", [NCH, 128, B], F32, kind="ExternalInput")
    i0_in = nc.dram_tensor("i0", [NCH, 128, B], F32, kind="ExternalInput")
    out_d = nc.dram_tensor("out", [T, 3, NCH, 128, B], F32, kind="ExternalOutput")

    stage = [nc.dram_tensor(f"stage{i}", [NCH, 128, B], F32) for i in range(2)]
    gath = [nc.dram_tensor(f"gath{i}", [KCH, 128, B], F32) for i in range(2)]

    with tile.TileContext(nc) as tc:
        with (
            tc.tile_pool(name="wpool", bufs=1) as wpool,
            tc.tile_pool(name="state", bufs=1) as spool,
            tc.tile_pool(name="work", bufs=3) as wk,
            tc.tile_pool(name="xp", bufs=6) as xp,
            tc.tile_pool(name="gp", bufs=2) as gp,
            tc.tile_pool(name="op", bufs=3) as op,
            tc.tile_pool(name="ps", bufs=2, space="PSUM") as ps,
            tc.tile_pool(name="ps2", bufs=2, space="PSUM") as ps2,
        ):
            w_sb = wpool.tile([128, KCH, NLOC], F32)
            f_sb = wpool.tile([128, NCH, NLOC], F32)
            js_sb = wpool.tile([128, B], F32)
            nc.scalar.dma_start(out=js_sb[:], in_=js_in.ap())
            cb_sb = wpool.tile([128, NCH, B], F32)
            cc_sb = wpool.tile([128, NCH, B], F32)
            cal_sb = wpool.tile([128, NCH, B], F32)
            nc.scalar.dma_start(out=w_sb[:], in_=w_in.ap().rearrange("k p n -> p k n"))
            nc.scalar.dma_start(out=f_sb[:], in_=f_in.ap().rearrange("k p n -> p k n"))
            nc.scalar.dma_start(out=cb_sb[:], in_=cb_in.ap().rearrange("k p b -> p k b"))
            nc.scalar.dma_start(out=cc_sb[:], in_=cc_in.ap().rearrange("k p b -> p k b"))
            nc.scalar.dma_start(out=cal_sb[:], in_=cal_in.ap().rearrange("k p b -> p k b"))

            u_sb = spool.tile([128, NCH, B], F32)
            i_sb = spool.tile([128, NCH, B], F32)
            nc.scalar.dma_start(out=u_sb[:], in_=u0_in.ap().rearrange("k p b -> p k b"))
            nc.scalar.dma_start(out=i_sb[:], in_=i0_in.ap().rearrange("k p b -> p k b"))

            for t in range(T):
                p = t % 2
                # prefetch this step's input column (ACT HWDGE ring)
                x_sb = xp.tile([128, NCH, 32], F32)
                nc.scalar.dma_start(
                    out=x_sb[:], in_=x_in.ap()[t].rearrange("k p b -> p k b")
                )

                # ---- membrane update + spikes (DVE) ----
                t1 = wk.tile([128, NCH, B], F32)
                t2 = wk.tile([128, NCH, B], F32)
                upre = wk.tile([128, NCH, B], F32)
                stage_all = op.tile([128, 3, NCH, B], F32)
                s_t = stage_all[:, 0]
                nc.vector.tensor_mul(t1[:], u_sb[:], cb_sb[:])
                nc.vector.tensor_mul(t2[:], i_sb[:], cc_sb[:])
                nc.vector.tensor_add(upre[:], t1[:], t2[:])
                nc.vector.tensor_scalar(
                    s_t, upre[:], THETA - U_REST, None, mybir.AluOpType.is_ge
                )
                # u after reset: keep where no spike
                nc.vector.scalar_tensor_tensor(
                    u_sb[:], s_t, 0.0, upre[:],
                    mybir.AluOpType.is_equal, mybir.AluOpType.mult,
                )

                # ---- stage spikes + AllGather + load gathered ----
                nc.sync.dma_start(
                    out=stage[p].ap().rearrange("k q b -> q k b"), in_=s_t
                )
                nc.gpsimd.collective_compute(
                    "AllGather", mybir.AluOpType.bypass, replica_groups=RG,
                    ins=[stage[p].ap().opt()], outs=[gath[p].ap().opt()],
                )
                # two half-gather loads: W rounds 0-3 only need chunks 0:16,
                # so the second half overlaps the first matmul rounds
                g_sb = gp.tile([128, KCH, B], F32)
                gview = gath[p].ap().rearrange("k q b -> q k b")
                nc.sync.dma_start(out=g_sb[:, 0:16], in_=gview[:, 0:16])
                nc.sync.dma_start(out=g_sb[:, 16:32], in_=gview[:, 16:32])

                # ---- matmuls, 4x column-tiled (one PE col-group per stripe j)
                # stripe j accumulates: feedforward chunk j + W chunks 8j..8j+7
                pb = ps.tile([128, NLOC], F32)
                for j in range(4):
                    nc.tensor.matmul(
                        pb[32 * j:32 * (j + 1), :], x_sb[:, j], f_sb[:, j],
                        start=True, stop=False, tile_position=(0, 32 * j),
                        skip_group_check=True,
                    )
                for m in range(8):
                    for j in range(4):
                        k = 4 * m + j  # round m touches chunks 4m..4m+3 only
                        nc.tensor.matmul(
                            pb[32 * j:32 * j + B, :], g_sb[:, k], w_sb[:, k],
                            start=False, stop=(m == 7), tile_position=(0, 32 * j),
                            skip_group_check=True,
                        )

                # ---- sum the 4 stripes with one matmul (lhsT = Jsel) ----
                s_copy = wk.tile([128, NLOC], F32)
                nc.vector.tensor_copy(s_copy[:], pb[:])
                pb2 = ps2.tile([32, NLOC], F32)
                nc.tensor.matmul(pb2[0:B, :], js_sb[:], s_copy[:],
                                 start=True, stop=True)

                # ---- transpose PSUM [8,512] back to state layout ----
                delta = wk.tile([128, NCH, 32], F32)
                pview = pb2[0:32, :].rearrange("q (c g s) -> q c g s", g=NCH, s=32)
                for g4 in range(4):
                    nc.vector.transpose(delta[32 * g4:32 * (g4 + 1)], pview[:, :, g4])

                # ---- I update ----
                t3 = wk.tile([128, NCH, B], F32)
                nc.vector.tensor_mul(t3[:], i_sb[:], cal_sb[:])
                nc.vector.tensor_add(i_sb[:], t3[:], delta[:, :, 0:B])

                # ---- stage outputs (I, v) and write out ----
                nc.scalar.copy(stage_all[:, 1], i_sb[:])
                nc.vector.tensor_scalar_add(stage_all[:, 2], u_sb[:], U_REST)
                nc.gpsimd.dma_start(
                    out=out_d.ap()[t].rearrange("c k q b -> q c k b"),
                    in_=stage_all[:],
                )
    nc.compile()
    return nc


class SpmdRunner:
    """jit once, execute many times; warm runs measure device time + dispatch."""

    def __init__(self, nc: bass.Bass, n_cores: int = NCORES):
        import jax
        from jax.sharding import Mesh, PartitionSpec
        from jax.experimental.shard_map import shard_map

        install_neuronx_cc_hook()
        self.jax = jax
        self.nc = nc
        self.n_cores = n_cores
        partition_name = nc.partition_id_tensor.name if nc.partition_id_tensor else None
        dbg_name = nc.dbg_addr.name if nc.dbg_addr else None
        in_names, out_names, out_avals, zero_outs = [], [], [], []
        for alloc in nc.m.functions[0].allocations:
            if not isinstance(alloc, mybir.MemoryLocationSet):
                continue
            name = alloc.memorylocations[0].name
            if alloc.kind == "ExternalInput":
                if name not in (partition_name, dbg_name):
                    in_names.append(name)
            elif alloc.kind == "ExternalOutput":
                out_names.append(name)
                shape = tuple(alloc.tensor_shape)
                dtype = mybir.dt.np(alloc.dtype)
                out_avals.append(jax.core.ShapedArray(shape, dtype))
                zero_outs.append(np.zeros(shape, dtype))
        self.in_names, self.out_names = in_names, out_names
        self.zero_outs = zero_outs
        n_params, n_outs = len(in_names), len(out_avals)
        all_in_names = list(in_names) + list(out_names)
        if dbg_name:
            all_in_names.append(dbg_name)
        if partition_name is not None:
            all_in_names.append(partition_name)

        def _body(*args):
            operands = list(args)
            if dbg_name:
                operands.append(jax.numpy.zeros((1, 2), jax.numpy.uint32))
            if partition_name is not None:
                operands.append(partition_id_tensor())
            outs = _bass_exec_p.bind(
                *operands,
                out_avals=tuple(out_avals),
                in_names=tuple(all_in_names),
                out_names=tuple(out_names),
                lowering_input_output_aliases=(),
                sim_require_finite=True,
                sim_require_nnan=True,
                nc=nc,
            )
            return tuple(outs)

        devices = jax.devices()[:n_cores]
        mesh = Mesh(np.asarray(devices), ("core",))
        in_specs = (PartitionSpec("core"),) * (n_params + n_outs)
        out_specs = (PartitionSpec("core"),) * n_outs
        self.fn = jax.jit(
            shard_map(_body, mesh=mesh, in_specs=in_specs, out_specs=out_specs,
                      check_rep=False),
            keep_unused=True,
        )

    def prepare(self, in_maps):
        from jax.sharding import Mesh, NamedSharding, PartitionSpec

        jax = self.jax
        concat = [
            np.concatenate([np.asarray(m[name]) for m in in_maps], axis=0)
            for name in self.in_names
        ]
        concat += [np.concatenate([z] * self.n_cores, axis=0) for z in self.zero_outs]
        mesh = Mesh(np.asarray(jax.devices()[:self.n_cores]), ("core",))
        sh = NamedSharding(mesh, PartitionSpec("core"))
        placed = [jax.device_put(a, sh) for a in concat]
        jax.block_until_ready(placed)
        return placed

    def run(self, concat_in):
        outs = self.fn(*concat_in)
        self.jax.block_until_ready(outs)
        return outs

    def split_outs(self, outs):
        results = [dict() for _ in range(self.n_cores)]
        for name, arr in zip(self.out_names, outs):
            arr = np.asarray(arr)
            per = arr.shape[0] // self.n_cores
            for c in range(self.n_cores):
                results[c][name] = arr[c * per:(c + 1) * per]
        return results


def _prep_inputs(neuron_types, recurrent_weights, feedforward_weights, inputs,
                 initial_v, initial_I, T):
    is_exc = (np.asarray(neuron_types) == 1)
    beta = np.where(is_exc, BETA_E, BETA_I).astype(np.float32)
    alpha = np.where(is_exc, ALPHA_E, ALPHA_I).astype(np.float32)
    Cgain = (R_E * (1.0 - beta)).astype(np.float32)
    scaling = np.where(is_exc, E_WEIGHT, I_WEIGHT).astype(np.float32)
    Wp = np.asarray(recurrent_weights, np.float32) * scaling[None, :]
    F = np.asarray(feedforward_weights, np.float32)
    xT = np.zeros((T, NIN, 32), np.float32)
    xT[:, :, :B] = np.asarray(inputs, np.float32).transpose(1, 2, 0)
    xT = xT.reshape(T, NCH, 128, 32)
    jsel = np.tile(np.eye(32, B, dtype=np.float32), (4, 1))  # (128, 8)
    u0 = np.ascontiguousarray(
        (np.asarray(initial_v, np.float32) - U_REST).T
    )  # (4096, 8)
    i0 = np.ascontiguousarray(np.asarray(initial_I, np.float32).T)

    def pc(vec):  # per-core const (4,128) -> (4,128,8)
        return np.ascontiguousarray(
            np.broadcast_to(vec.reshape(NCORES, NCH, 128, 1), (NCORES, NCH, 128, B))
        ).astype(np.float32)

    cb, cal, ccg = pc(beta), pc(alpha), pc(Cgain)
    in_maps = []
    for c in range(NCORES):
        cols = slice(c * NLOC, (c + 1) * NLOC)
        in_maps.append({
            "w": np.ascontiguousarray(Wp[:, cols]).reshape(KCH, 128, NLOC),
            "f": np.ascontiguousarray(F[:, cols]).reshape(NCH, 128, NLOC),
            "x": xT,
            "jsel": jsel,
            "cb": cb[c], "cc": ccg[c], "cal": cal[c],
            "u0": u0[cols].reshape(NCH, 128, B),
            "i0": i0[cols].reshape(NCH, 128, B),
        })
    return in_maps


def _unshard(results, T):
    outs = []
    for ch in range(3):
        parts = []
        for c in range(NCORES):
            a = results[c]["out"][:, ch]  # (T, 4, 128, 8)
            parts.append(a.reshape(T, NLOC, B).transpose(2, 0, 1))
        outs.append(np.concatenate(parts, axis=2))  # (8, T, 4096)
    return outs[0], outs[2], outs[1]  # s, v, I


_CACHE = {}


def _get_runner(T):
    if T not in _CACHE:
        nc = build_lif_kernel(T)
        _CACHE[T] = SpmdRunner(nc, NCORES)
    return _CACHE[T]


def kernel(neuron_types, recurrent_weights, feedforward_weights, inputs,
           initial_v, initial_I, n_steps):
    T = int(n_steps)
    assert np.asarray(inputs).shape == (B, T, NIN)
    in_maps = _prep_inputs(neuron_types, recurrent_weights, feedforward_weights,
                           inputs, initial_v, initial_I, T)
    runner = _get_runner(T)
    concat_in = runner.prepare(in_maps)
    outs = runner.run(concat_in)
    results = runner.split_outs(outs)
    s, v, I = _unshard(results, T)
    return s, v, I
